# revision 13
# baseline (speedup 1.0000x reference)
"""FocalLoss + MDCA loss kernel for TRN2, 8-core data-parallel. v6.

reference:
    loss_cls = mean_i[-(1-pt_i) * log(pt_i)],  pt_i = probs[i, targets[i]]
    loss_cal = mean_c |mean_i probs[i,c] - count_c/B|
    out = loss_cls + loss_cal        (GAMMA=1, BETA=1)

Strategy: shard batch (16384) across 8 cores (2048 rows each). The SWDGE
stream of the 8.2 MB probs shard dominates; measurements show it is DMA-
engine/write-side paced (~180 GB/s write-side for cast DMAs), so the colsum
copy converts f32 -> f8e5 (e5m2) in flight: half the write bytes of fp16.
probs ~1e-3 sit in e5m2's normal range (min normal 2^-14); the ~12%/elem
quantization averages out over 2048 rows and only touches loss_cal
(~6% of the total loss) -> ~1e-3 relative error on the loss, 20x inside
the 2e-2 gate. pt for the focal term comes from an EXACT fp32 indirect
gather, and the histogram from exact 0/1 fp16 one-hots, so neither is
touched by fp8.

  - probs: seven [128,2000] f8e5 big-tiles (256 rows: partition p holds
    rows 256k+2p, 256k+2p+1; 8000 B contiguous read descriptors) + TWO
    [128,1000] half-tiles for the last 256 rows so the final DMA gates
    only 2 tail matmuls. gpsimd emits ONLY DMAs (plus one tiny iota):
    k0,k1,k2, rowbase-iota, pt-gather, k3..k7b - the queue never starves
    and the gather's 2048 tiny descriptors execute mid-stream.
  - column sums: ones^T @ tile chunks, 32 fp8 matmuls into 2 PSUM banks.
  - histogram: rank-2 factorization c = 128a+b. eqA[p,:]=(iota8==a(t)),
    eqB[p,:]=(iota128==b(t)); PSUM hist2d[8,128] += eqA^T @ eqB per
    128-row group: 16 tiny fp16 matmuls that depend only on the early
    HWDGE targets load; they warm the PE clock and drain mid-stream via
    their own [8,128] output DMA. Exact counts.
  - pt: ONE indirect gather (offsets = 1000*row + t built from a tiny
    gpsimd iota + DVE add), exact fp32. It lands mid-stream, so the whole
    focal chain (ACT [pt|ln pt], DVE (pt-1)*ln(pt) row-fold) is hidden;
    ones_f32^T @ focal folds partitions into a [1,1] PSUM scalar.
  - targets arrive pre-arranged via ONE strided HWDGE descriptor set:
    t_bt[p, 2k+j] = targets[256k+2p+j] (no PE transpose, no identity).
  - tail after the last half-tile packet: 2 fp8 matmuls -> stage
    [colsum | focal] -> one [1,1001] DMA. Host combines cores: colsum /
    hist2d all-reduce + focal sum, then the loss formula.

The walrus build in this env encodes at most ONE sync wait per instruction;
_split_multi_waits post-processes the scheduled program to hoist extra waits
onto same-engine EventSemaphore carriers. _compact_sem_ids densely remaps
semaphores to ids 3.. and --max-sem-num caps the allocator.
"""

import numpy as np

import concourse.bass as bass
import concourse.bass_utils as _bu
import concourse.mybir as mybir
import concourse.tile as tile
from concourse.bass_utils import run_bass_kernel_spmd

if not getattr(_bu.bir_verify_and_optimise, "_sem_capped", False):
    _orig_bvo = _bu.bir_verify_and_optimise

    def _patch_neff_rtsem(neff_path):
        """Optionally raise def.json's runtime_semaphore_count. The runtime's
        end-of-NEFF sweep clears every semaphore id EXCEPT the first
        runtime_semaphore_count — raising it shrinks the ~250-instruction
        per-id clear loop the runtime appends to the engine streams. Our
        program's own EVENT_SEMAPHORE_RANGE_CLEAR already zeroes the ids it
        used, so a re-execution still starts clean."""
        import io as _io
        import os as _os
        import tarfile as _tarfile
        import tempfile as _tempfile

        import orjson as _orjson

        from concourse.neff import make_deterministic_neff_header

        val = _os.environ.get("KERNEL_RT_SEM_COUNT", "")
        if not val:
            return
        with _tempfile.TemporaryDirectory() as rd:
            with open(neff_path, "rb") as f:
                old_header = f.read(1024)
                with _tarfile.open(fileobj=f, mode="r") as t:
                    t.extractall(rd)
            p = f"{rd}/sg00/def.json"
            d = _orjson.loads(open(p, "rb").read())
            d["runtime_semaphore_count"] = int(val)
            open(p, "wb").write(_orjson.dumps(d))
            buf = _io.BytesIO()

            def _reset(ti):
                ti.mtime = 0
                ti.uid = 0
                ti.gid = 0
                ti.uname = "nobody"
                ti.gname = "nobody"
                return ti

            with _tarfile.open(fileobj=buf, mode="w") as t:
                t.add(rd, arcname=".", filter=_reset)
            data = buf.getvalue()
            header = make_deterministic_neff_header(
                old_neff_header=old_header, new_neff_data=data)
        with open(neff_path, "wb") as f:
            f.write(header + data)

    def _bvo_capped(*args, **kwargs):
        import concourse.bass_utils as bu

        orig_run = bu.run_command

        def run_with_cap(cmd, **kw):
            if any("codegen" in str(c) for c in cmd):
                cmd = list(cmd) + ["--max-sem-num=32"]
                import os as _os
                extra = _os.environ.get("KERNEL_WALRUS_EXTRA", "")
                if extra:
                    cmd = cmd + extra.split()
            return orig_run(cmd, **kw)

        bu.run_command = run_with_cap
        try:
            ret = _orig_bvo(*args, **kwargs)
        finally:
            bu.run_command = orig_run
        if isinstance(ret, str):
            try:
                _patch_neff_rtsem(ret)
            except Exception as e:
                print(f"neff rtsem patch skipped: {e}")
        return ret

    _bvo_capped._sem_capped = True
    _bu.bir_verify_and_optimise = _bvo_capped

B, C = 16384, 1000
NCORES = 8
BC = B // NCORES  # 2048 rows per core
P = 128
NBF = 7           # full big-tiles per core: [128, 2000], 256 rows each
J = 2             # rows per partition per full big-tile
W = J * C         # 2000 columns per big-tile
NG = 16           # 128-row groups per core (pt / hist granularity)
CH = 500          # matmul chunk free-dim (PSUM bank = 512 fp32)
NWU = 4           # PE warm-up matmuls
OUT_W = 1001      # [colsum 0:1000 | focal_sum]
HA, HB = 8, 128   # hist2d factorization: class c = 128*a + b

F32 = mybir.dt.float32
F16 = mybir.dt.float16
F8 = mybir.dt.float8e5
I32 = mybir.dt.int32


def emit_kernel(ctx, tc, probs_d, targ_d, out_d, hist_d):
    nc = tc.nc
    Alu = mybir.AluOpType
    Act = mybir.ActivationFunctionType

    consts = ctx.enter_context(tc.tile_pool(name="consts", bufs=1))
    probs_pool = ctx.enter_context(tc.tile_pool(name="probs_pool", bufs=NBF + 4))
    eq_pool = ctx.enter_context(tc.tile_pool(name="eq_pool", bufs=NG))
    psum = ctx.enter_context(tc.tile_pool(name="psum", bufs=1, space="PSUM"))

    # 1) targets first: ONE strided HWDGE load lands t_bt[p, 2k+j] =
    # targets[256k+2p+j] (descriptor: 8 chunks x 8 B, stride 1 KiB).
    t_bt_i32 = consts.tile([P, NG], I32, tag="t_bt_i32")
    nc.sync.dma_start(
        out=t_bt_i32[:],
        in_=targ_d.rearrange("(k p j) -> p k j", k=NBF + 1, p=P, j=J),
    )

    # 2) probs stream, f32 -> f8e5 in flight (write-side is the DMA pacing
    # constraint; e5m2 halves it vs fp16).
    def load_full(k):
        pf8 = probs_pool.tile([P, W], F8, tag="pf8", name=f"pf8_{k}")
        nc.gpsimd.dma_start(
            out=pf8[:],
            in_=probs_d[k * J * P:(k + 1) * J * P, :].rearrange(
                "(p j) c -> p (j c)", p=P, j=J),
        )
        return pf8

    def load_quarter(qq):
        pf8 = probs_pool.tile([P, CH], F8, tag="pf8", name=f"pf8_7{'abcd'[qq]}")
        nc.gpsimd.dma_start(
            out=pf8[:],
            in_=probs_d[NBF * J * P:(NBF + 1) * J * P, :].rearrange(
                "(p j) c -> p (j c)", p=P, j=J)[:, qq * CH:(qq + 1) * CH],
        )
        return pf8

    # tiny [128,1] p-index iota first on gpsimd (~0.1 us): feeds the
    # block-ones lhsT below long before the first colsum matmul.
    chi = consts.tile([P, 1], I32, tag="chi")
    nc.gpsimd.iota(chi[:], pattern=[[0, 1]], base=0, channel_multiplier=1)

    pf8s = [load_full(k) for k in range(3)]

    # 3) rowbase iota + pt gather emissions sit between k2 and k3 so the
    # 2048 tiny descriptors execute mid-stream, and offs (needs the HWDGE
    # targets, landing ~2 us in) is ready just in time.
    rowidx = consts.tile([P, NG], I32, tag="rowidx")
    nc.gpsimd.iota(rowidx[:], pattern=[[J * P, NBF + 1], [1, J]], base=0,
                   channel_multiplier=J)
    offs = consts.tile([P, NG], I32, tag="offs")
    nc.vector.tensor_scalar(out=offs[:], in0=rowidx[:], scalar1=float(C),
                            scalar2=None, op0=Alu.mult)
    nc.vector.tensor_tensor(out=offs[:], in0=offs[:], in1=t_bt_i32[:],
                            op=Alu.add)
    pt_all = consts.tile([P, NG], F32, tag="pt_all")
    nc.gpsimd.indirect_dma_start(
        out=pt_all[:], out_offset=None,
        in_=probs_d.rearrange("a b -> (a b)")[:, None],
        in_offset=bass.IndirectOffsetOnAxis(ap=offs[:], axis=0),
    )

    pf8s += [load_full(k) for k in range(3, NBF)]
    pf8s += [load_quarter(qq) for qq in range(4)]

    # 4) constants on DVE: iota128 by prefix-scan, ones in three dtypes.
    ones128 = consts.tile([P, HB], F16, tag="ones128")
    nc.vector.memset(ones128[:], 1.0)
    ones_f8 = consts.tile([P, 1], F8, tag="ones_f8")
    nc.vector.memset(ones_f8[:], 1.0)
    ones_f32 = consts.tile([P, 1], F32, tag="ones_f32")
    nc.vector.memset(ones_f32[:], 1.0)
    wu_f8 = consts.tile([P, CH], F8, tag="wu_f8")
    nc.vector.memset(wu_f8[:], 0.5)
    iota128 = consts.tile([P, HB], F16, tag="iota128")
    nc.vector.tensor_tensor_scan(
        out=iota128[:], data0=ones128[:], data1=ones128[:],
        initial=-1.0, op0=Alu.add, op1=Alu.bypass,
    )
    # block-ones lhsT: ones4[p, c] = (c == p>>5) so colsum lands as [4,500]
    # per bank (partition-parallel staging copies, host sums the 4 rows).
    chi5 = consts.tile([P, 1], I32, tag="chi5")
    nc.vector.tensor_scalar(out=chi5[:], in0=chi[:], scalar1=5, scalar2=None,
                            op0=Alu.arith_shift_right)
    chi5f = consts.tile([P, 1], F32, tag="chi5f")
    nc.vector.tensor_copy(chi5f[:], chi5[:])
    ones4 = consts.tile([P, 4], F8, tag="ones4")
    nc.vector.tensor_scalar(out=ones4[:], in0=iota128[:, 0:4],
                            scalar1=chi5f[:], scalar2=None, op0=Alu.is_equal)

    # 5) PE warm-up on the fp8 path while the first tiles stream in.
    wu_ps = psum.tile([1, CH], F32, tag="wu_ps")
    for w in range(NWU):
        nc.tensor.matmul(wu_ps[:], ones_f8[:], wu_f8[:],
                         start=(w == 0), stop=(w == NWU - 1))

    # 6) a = t>>7, b = t&127 (f32 for the eq compares).
    a_i32 = consts.tile([P, NG], I32, tag="a_i32")
    nc.vector.tensor_scalar(out=a_i32[:], in0=t_bt_i32[:], scalar1=7,
                            scalar2=None, op0=Alu.arith_shift_right)
    b_i32 = consts.tile([P, NG], I32, tag="b_i32")
    nc.vector.tensor_scalar(out=b_i32[:], in0=t_bt_i32[:], scalar1=127,
                            scalar2=None, op0=Alu.bitwise_and)
    a_f32 = consts.tile([P, NG], F32, tag="a_f32")
    nc.vector.tensor_copy(a_f32[:], a_i32[:])
    b_f32 = consts.tile([P, NG], F32, tag="b_f32")
    nc.vector.tensor_copy(b_f32[:], b_i32[:])

    # 7) histogram: hist2d[a,b] += eqA_i^T @ eqB_i per 128-row group.
    hist_ps = psum.tile([HA, HB], F32, tag="hist_ps")
    for i in range(NG):
        eqA = eq_pool.tile([P, HA], F16, tag="eqA", name=f"eqA_{i}")
        nc.vector.tensor_scalar(out=eqA[:], in0=iota128[:, 0:HA],
                                scalar1=a_f32[:, i:i + 1], scalar2=None,
                                op0=Alu.is_equal)
        eqB = eq_pool.tile([P, HB], F16, tag="eqB", name=f"eqB_{i}")
        nc.vector.tensor_scalar(out=eqB[:], in0=iota128[:],
                                scalar1=b_f32[:, i:i + 1], scalar2=None,
                                op0=Alu.is_equal)
        nc.tensor.matmul(hist_ps[:], eqA[:], eqB[:],
                         start=(i == 0), stop=(i == NG - 1))

    # hist2d drains mid-stream: ACT stage + its own small HWDGE DMA.
    hist_sb = consts.tile([HA, HB], F32, tag="hist_sb")
    nc.scalar.copy(hist_sb[:], hist_ps[:])
    nc.sync.dma_start(out=hist_d[:, :], in_=hist_sb[:])

    # 8) DMA-paced colsum matmuls: 4 fp8 chunks per full tile, banks
    # alternate so bank0 = classes 0:500, bank1 = 500:1000.
    cs_ps = [psum.tile([4, CH], F32, tag=f"cs_ps{h}", name=f"cs_ps{h}")
             for h in range(2)]
    for k in range(NBF):
        for q in range(2 * J):
            sl = slice(q * CH, (q + 1) * CH)
            nc.tensor.matmul(cs_ps[q % 2][:], ones4[:], pf8s[k][:, sl],
                             start=(k == 0 and q < 2), stop=False)

    # 9) focal chain, fully hidden mid-stream (pt lands ~mid-stream):
    # pl = [pt | ln pt], focal[p] = sum_m (pt-1)*ln(pt), PE-fold to [1,1].
    pl = consts.tile([P, 2 * NG], F32, tag="pl")
    nc.scalar.copy(pl[:, 0:NG], pt_all[:])
    nc.scalar.activation(pl[:, NG:2 * NG], pt_all[:], Act.Ln)
    junk2 = consts.tile([P, NG], F32, tag="junk2")
    focal = consts.tile([P, 1], F32, tag="focal")
    nc.vector.scalar_tensor_tensor(
        out=junk2[:], in0=pl[:, 0:NG], scalar=1.0, in1=pl[:, NG:2 * NG],
        op0=Alu.subtract, op1=Alu.mult, accum_out=focal[:],
    )

    # 10) tail: four quarter-tiles -> 1 matmul each (bank0 closes at 7c,
    # bank1 at 7d); staging copies are [4,500] partition-parallel; the
    # final [4,1001] DMA rides the hot SWDGE ring.
    out_sb = consts.tile([4, OUT_W], F32, tag="out_sb")
    for qq in range(4):
        nc.tensor.matmul(cs_ps[qq % 2][:], ones4[:], pf8s[NBF + qq][:],
                         start=False, stop=(qq >= 2))
        if qq == 2:
            nc.vector.tensor_copy(out_sb[:, 0:CH], cs_ps[0][:])
    fc_ps = psum.tile([1, 1], F32, tag="fc_ps")
    nc.tensor.matmul(fc_ps[:], ones_f32[:], focal[:], start=True, stop=True)

    nc.scalar.copy(out_sb[:, CH:2 * CH], cs_ps[1][:])
    nc.scalar.copy(out_sb[0:1, 2 * CH:OUT_W], fc_ps[:])
    nc.gpsimd.dma_start(out=out_d[:, :], in_=out_sb[:])


def _split_multi_waits(nc):
    """The walrus build in this env encodes at most ONE sync wait per
    instruction (newer Tile emits several, e.g. on its tail drain). Hoist
    extra waits onto EventSemaphore carrier instructions inserted just
    before, on the same engine — same-engine program order makes this
    semantically identical."""
    n = 0
    for f in nc.m.functions:
        for blk in f.blocks:
            il = blk.instructions
            i = 0
            while i < len(il):
                inst = il[i]
                si = inst.sync_info
                ws = list(si.on_wait) if si is not None else []
                if len(ws) > 1:
                    for w in ws[:-1]:
                        ev = mybir.InstEventSemaphore(
                            name=f"I-waitsplit-{n}", ins=[], outs=[])
                        n += 1
                        ev.engine = inst.engine
                        ev.sync_info = mybir.SyncInfo(on_wait=[w], on_update=[])
                        il.insert(i, ev)
                        i += 1
                    inst.sync_info = mybir.SyncInfo(
                        on_wait=[ws[-1]], on_update=list(si.on_update))
                i += 1


def _compact_sem_ids(nc, base=3):
    """Tile/bass allocate semaphore ids from ~151 up; remap every semaphore
    this program touches down to [base, base+n) so the program sits inside
    a small --max-sem-num cap. ids 0-2 stay free for the compiler's own
    barriers."""
    def insts():
        for f in nc.m.functions:
            for b in f.blocks:
                yield from b.instructions

    used = set()
    for inst in insts():
        si = inst.sync_info
        if si:
            for w in list(si.on_wait):
                if w.sync_type == "semaphore":
                    used.add(w.id)
            for u in list(si.on_update):
                if u.sync_type == "semaphore":
                    used.add(u.id)
    m = {old: base + i for i, old in enumerate(sorted(used))}
    for inst in insts():
        si = inst.sync_info
        if si:
            ws, us = list(si.on_wait), list(si.on_update)
            changed = False
            for w in ws:
                if w.sync_type == "semaphore" and w.id in m:
                    w.id = m[w.id]
                    changed = True
            for u in us:
                if u.sync_type == "semaphore" and u.id in m:
                    u.id = m[u.id]
                    changed = True
            if changed:
                inst.sync_info = mybir.SyncInfo(on_wait=ws, on_update=us)
        if (type(inst).__name__ == "InstISA"
                and getattr(inst, "op_name", "") == "EVENT_SEMAPHORE_RANGE_CLEAR"):
            d = inst.ant_dict
            ids = [m[x] for x in range(d["range_first"], d["range_last"] + 1)
                   if x in m]
            nf, nl = (min(ids), max(ids)) if ids else (base, base)
            d["range_first"], d["range_last"] = nf, nl
            v = list(inst.instr)
            v[13], v[14] = nf, nl
            inst.instr = v
            inst.ant_dict = d


_cached_nc = {}


def build_nc(split_waits=True):
    global _cached_nc
    if split_waits in _cached_nc:
        return _cached_nc[split_waits]
    from contextlib import ExitStack

    nc = bass.Bass("TRN2", dynamic_dma_scratch_size=131072)
    probs_d = nc.dram_tensor("probs", [BC, C], F32, kind="ExternalInput").ap()
    targ_d = nc.dram_tensor("targets", [BC], I32, kind="ExternalInput").ap()
    out_d = nc.dram_tensor("out_all", [4, OUT_W], F32, kind="ExternalOutput").ap()
    hist_d = nc.dram_tensor("out_hist", [HA, HB], F32, kind="ExternalOutput").ap()

    with tile.TileContext(nc) as tc:
        with ExitStack() as ctx:
            emit_kernel(ctx, tc, probs_d, targ_d, out_d, hist_d)
    if split_waits:
        _split_multi_waits(nc)
    _compact_sem_ids(nc)
    _cached_nc[split_waits] = nc
    return nc


def make_in_maps(probs, targets):
    probs = np.ascontiguousarray(np.asarray(probs), dtype=np.float32)
    targets = np.asarray(targets).astype(np.int32)
    assert probs.shape == (B, C) and targets.shape == (B,)
    return [
        {
            "probs": probs[k * BC:(k + 1) * BC],
            "targets": np.ascontiguousarray(targets[k * BC:(k + 1) * BC]),
        }
        for k in range(NCORES)
    ]


def combine(results):
    cs = np.zeros(C, np.float64)
    hs = np.zeros(C, np.float64)
    fc = 0.0
    for r in results:
        rows = r["out_all"].reshape(4, OUT_W).astype(np.float64)
        cs[0:CH] += rows[:, 0:CH].sum(axis=0)
        cs[CH:C] += rows[:, CH:C].sum(axis=0)
        fc += rows[0, C]
        hs += r["out_hist"].reshape(HA * HB).astype(np.float64)[0:C]
    loss_cls = fc / B
    loss_cal = float(np.mean(np.abs(cs / B - hs / B)))
    return np.asarray(loss_cls + 1.0 * loss_cal, dtype=np.float32)


def run_spmd(probs, targets, **kwargs):
    nc = build_nc()
    in_maps = make_in_maps(probs, targets)
    return run_bass_kernel_spmd(nc, in_maps, list(range(NCORES)), **kwargs)


def kernel(probs, targets):
    res = run_spmd(probs, targets)
    return combine(res.results)


# revision 14
# speedup vs baseline: 1.2057x; 1.2057x over previous
"""FocalLoss + MDCA loss kernel for TRN2, 8-core data-parallel. v6.

reference:
    loss_cls = mean_i[-(1-pt_i) * log(pt_i)],  pt_i = probs[i, targets[i]]
    loss_cal = mean_c |mean_i probs[i,c] - count_c/B|
    out = loss_cls + loss_cal        (GAMMA=1, BETA=1)

Strategy: shard batch (16384) across 8 cores (2048 rows each). The SWDGE
stream of the 8.2 MB probs shard dominates; measurements show it is DMA-
engine/write-side paced (~180 GB/s write-side for cast DMAs), so the colsum
copy converts f32 -> f8e5 (e5m2) in flight: half the write bytes of fp16.
probs ~1e-3 sit in e5m2's normal range (min normal 2^-14); the ~12%/elem
quantization averages out over 2048 rows and only touches loss_cal
(~6% of the total loss) -> ~1e-3 relative error on the loss, 20x inside
the 2e-2 gate. pt for the focal term comes from an EXACT fp32 indirect
gather, and the histogram from exact 0/1 fp16 one-hots, so neither is
touched by fp8.

  - probs: seven [128,2000] f8e5 big-tiles (256 rows: partition p holds
    rows 256k+2p, 256k+2p+1; 8000 B contiguous read descriptors) + TWO
    [128,1000] half-tiles for the last 256 rows so the final DMA gates
    only 2 tail matmuls. gpsimd emits ONLY DMAs (plus one tiny iota):
    k0,k1,k2, rowbase-iota, pt-gather, k3..k7b - the queue never starves
    and the gather's 2048 tiny descriptors execute mid-stream.
  - column sums: ones^T @ tile chunks, 32 fp8 matmuls into 2 PSUM banks.
  - histogram: rank-2 factorization c = 128a+b. eqA[p,:]=(iota8==a(t)),
    eqB[p,:]=(iota128==b(t)); PSUM hist2d[8,128] += eqA^T @ eqB per
    128-row group: 16 tiny fp16 matmuls that depend only on the early
    HWDGE targets load; they warm the PE clock and drain mid-stream via
    their own [8,128] output DMA. Exact counts.
  - pt: ONE indirect gather (offsets = 1000*row + t built from a tiny
    gpsimd iota + DVE add), exact fp32. It lands mid-stream, so the whole
    focal chain (ACT [pt|ln pt], DVE (pt-1)*ln(pt) row-fold) is hidden;
    ones_f32^T @ focal folds partitions into a [1,1] PSUM scalar.
  - targets arrive pre-arranged via ONE strided HWDGE descriptor set:
    t_bt[p, 2k+j] = targets[256k+2p+j] (no PE transpose, no identity).
  - tail after the last half-tile packet: 2 fp8 matmuls -> stage
    [colsum | focal] -> one [1,1001] DMA. Host combines cores: colsum /
    hist2d all-reduce + focal sum, then the loss formula.

The walrus build in this env encodes at most ONE sync wait per instruction;
_split_multi_waits post-processes the scheduled program to hoist extra waits
onto same-engine EventSemaphore carriers. _compact_sem_ids densely remaps
semaphores to ids 3.. and --max-sem-num caps the allocator.
"""

import numpy as np

import concourse.bass as bass
import concourse.bass_utils as _bu
import concourse.mybir as mybir
import concourse.tile as tile
from concourse.bass_utils import run_bass_kernel_spmd

if not getattr(_bu.bir_verify_and_optimise, "_sem_capped", False):
    _orig_bvo = _bu.bir_verify_and_optimise

    def _patch_neff_rtsem(neff_path):
        """Optionally raise def.json's runtime_semaphore_count. The runtime's
        end-of-NEFF sweep clears every semaphore id EXCEPT the first
        runtime_semaphore_count — raising it shrinks the ~250-instruction
        per-id clear loop the runtime appends to the engine streams. Our
        program's own EVENT_SEMAPHORE_RANGE_CLEAR already zeroes the ids it
        used, so a re-execution still starts clean."""
        import io as _io
        import os as _os
        import tarfile as _tarfile
        import tempfile as _tempfile

        import orjson as _orjson

        from concourse.neff import make_deterministic_neff_header

        val = _os.environ.get("KERNEL_RT_SEM_COUNT", "")
        if not val:
            return
        with _tempfile.TemporaryDirectory() as rd:
            with open(neff_path, "rb") as f:
                old_header = f.read(1024)
                with _tarfile.open(fileobj=f, mode="r") as t:
                    t.extractall(rd)
            p = f"{rd}/sg00/def.json"
            d = _orjson.loads(open(p, "rb").read())
            d["runtime_semaphore_count"] = int(val)
            open(p, "wb").write(_orjson.dumps(d))
            buf = _io.BytesIO()

            def _reset(ti):
                ti.mtime = 0
                ti.uid = 0
                ti.gid = 0
                ti.uname = "nobody"
                ti.gname = "nobody"
                return ti

            with _tarfile.open(fileobj=buf, mode="w") as t:
                t.add(rd, arcname=".", filter=_reset)
            data = buf.getvalue()
            header = make_deterministic_neff_header(
                old_neff_header=old_header, new_neff_data=data)
        with open(neff_path, "wb") as f:
            f.write(header + data)

    def _bvo_capped(*args, **kwargs):
        import concourse.bass_utils as bu

        orig_run = bu.run_command

        def run_with_cap(cmd, **kw):
            if any("codegen" in str(c) for c in cmd):
                cmd = list(cmd) + ["--max-sem-num=32"]
                import os as _os
                extra = _os.environ.get("KERNEL_WALRUS_EXTRA", "")
                if extra:
                    cmd = cmd + extra.split()
            return orig_run(cmd, **kw)

        bu.run_command = run_with_cap
        try:
            ret = _orig_bvo(*args, **kwargs)
        finally:
            bu.run_command = orig_run
        if isinstance(ret, str):
            try:
                _patch_neff_rtsem(ret)
            except Exception as e:
                print(f"neff rtsem patch skipped: {e}")
        return ret

    _bvo_capped._sem_capped = True
    _bu.bir_verify_and_optimise = _bvo_capped

B, C = 16384, 1000
NCORES = 8
BC = B // NCORES  # 2048 rows per core
P = 128
NBF = 7           # full big-tiles per core: [128, 2000], 256 rows each
J = 2             # rows per partition per full big-tile
W = J * C         # 2000 columns per big-tile
NG = 16           # 128-row groups per core (pt / hist granularity)
CH = 500          # matmul chunk free-dim (PSUM bank = 512 fp32)
NWU = 4           # PE warm-up matmuls
OUT_W = 1001      # [colsum 0:1000 | focal_sum]
HA, HB = 8, 128   # hist2d factorization: class c = 128*a + b

F32 = mybir.dt.float32
F16 = mybir.dt.float16
F8 = mybir.dt.float8e5
I32 = mybir.dt.int32


def emit_kernel(ctx, tc, probs_d, targ_d, out_d, hist_d):
    nc = tc.nc
    Alu = mybir.AluOpType
    Act = mybir.ActivationFunctionType

    consts = ctx.enter_context(tc.tile_pool(name="consts", bufs=1))
    probs_pool = ctx.enter_context(tc.tile_pool(name="probs_pool", bufs=NBF + 2))
    eq_pool = ctx.enter_context(tc.tile_pool(name="eq_pool", bufs=NG))
    psum = ctx.enter_context(tc.tile_pool(name="psum", bufs=1, space="PSUM"))

    # 1) targets first: ONE strided HWDGE load lands t_bt[p, 2k+j] =
    # targets[256k+2p+j] (descriptor: 8 chunks x 8 B, stride 1 KiB).
    t_bt_i32 = consts.tile([P, NG], I32, tag="t_bt_i32")
    nc.sync.dma_start(
        out=t_bt_i32[:],
        in_=targ_d.rearrange("(k p j) -> p k j", k=NBF + 1, p=P, j=J),
    )

    # 2) probs stream, f32 -> f8e5 in flight (write-side is the DMA pacing
    # constraint; e5m2 halves it vs fp16).
    def load_full(k):
        pf8 = probs_pool.tile([P, W], F8, tag="pf8", name=f"pf8_{k}")
        nc.gpsimd.dma_start(
            out=pf8[:],
            in_=probs_d[k * J * P:(k + 1) * J * P, :].rearrange(
                "(p j) c -> p (j c)", p=P, j=J),
        )
        return pf8

    def load_half(h):
        pf8 = probs_pool.tile([P, C], F8, tag="pf8", name=f"pf8_7{'ab'[h]}")
        nc.gpsimd.dma_start(
            out=pf8[:],
            in_=probs_d[NBF * J * P:(NBF + 1) * J * P, :].rearrange(
                "(p j) c -> p (j c)", p=P, j=J)[:, h * C:(h + 1) * C],
        )
        return pf8

    # tiny [128,1] p-index iota first on gpsimd (~0.1 us): feeds the
    # block-ones lhsT below long before the first colsum matmul.
    chi = consts.tile([P, 1], I32, tag="chi")
    nc.gpsimd.iota(chi[:], pattern=[[0, 1]], base=0, channel_multiplier=1)

    pf8s = [load_full(k) for k in range(3)]

    # 3) rowbase iota + pt gather emissions sit between k2 and k3 so the
    # 2048 tiny descriptors execute mid-stream, and offs (needs the HWDGE
    # targets, landing ~2 us in) is ready just in time.
    rowidx = consts.tile([P, NG], I32, tag="rowidx")
    nc.gpsimd.iota(rowidx[:], pattern=[[J * P, NBF + 1], [1, J]], base=0,
                   channel_multiplier=J)
    offs = consts.tile([P, NG], I32, tag="offs")
    nc.vector.tensor_scalar(out=offs[:], in0=rowidx[:], scalar1=float(C),
                            scalar2=None, op0=Alu.mult)
    nc.vector.tensor_tensor(out=offs[:], in0=offs[:], in1=t_bt_i32[:],
                            op=Alu.add)
    pt_all = consts.tile([P, NG], F32, tag="pt_all")
    nc.gpsimd.indirect_dma_start(
        out=pt_all[:], out_offset=None,
        in_=probs_d.rearrange("a b -> (a b)")[:, None],
        in_offset=bass.IndirectOffsetOnAxis(ap=offs[:], axis=0),
    )

    pf8s += [load_full(k) for k in range(3, NBF)]
    pf8s += [load_half(0), load_half(1)]

    # 4) constants on DVE: iota128 by prefix-scan, ones in three dtypes.
    ones128 = consts.tile([P, HB], F16, tag="ones128")
    nc.vector.memset(ones128[:], 1.0)
    ones_f8 = consts.tile([P, 1], F8, tag="ones_f8")
    nc.vector.memset(ones_f8[:], 1.0)
    ones_f32 = consts.tile([P, 1], F32, tag="ones_f32")
    nc.vector.memset(ones_f32[:], 1.0)
    wu_f8 = consts.tile([P, CH], F8, tag="wu_f8")
    nc.vector.memset(wu_f8[:], 0.5)
    iota128 = consts.tile([P, HB], F16, tag="iota128")
    nc.vector.tensor_tensor_scan(
        out=iota128[:], data0=ones128[:], data1=ones128[:],
        initial=-1.0, op0=Alu.add, op1=Alu.bypass,
    )
    # block-ones lhsT: ones4[p, c] = (c == p>>5) so colsum lands as [4,500]
    # per bank (partition-parallel staging copies, host sums the 4 rows).
    chi5 = consts.tile([P, 1], I32, tag="chi5")
    nc.vector.tensor_scalar(out=chi5[:], in0=chi[:], scalar1=5, scalar2=None,
                            op0=Alu.arith_shift_right)
    chi5f = consts.tile([P, 1], F32, tag="chi5f")
    nc.vector.tensor_copy(chi5f[:], chi5[:])
    ones4 = consts.tile([P, 4], F8, tag="ones4")
    nc.vector.tensor_scalar(out=ones4[:], in0=iota128[:, 0:4],
                            scalar1=chi5f[:], scalar2=None, op0=Alu.is_equal)

    # 5) PE warm-up on the fp8 path while the first tiles stream in.
    wu_ps = psum.tile([1, CH], F32, tag="wu_ps")
    for w in range(NWU):
        nc.tensor.matmul(wu_ps[:], ones_f8[:], wu_f8[:],
                         start=(w == 0), stop=(w == NWU - 1))

    # 6) a = t>>7, b = t&127 (f32 for the eq compares).
    a_i32 = consts.tile([P, NG], I32, tag="a_i32")
    nc.vector.tensor_scalar(out=a_i32[:], in0=t_bt_i32[:], scalar1=7,
                            scalar2=None, op0=Alu.arith_shift_right)
    b_i32 = consts.tile([P, NG], I32, tag="b_i32")
    nc.vector.tensor_scalar(out=b_i32[:], in0=t_bt_i32[:], scalar1=127,
                            scalar2=None, op0=Alu.bitwise_and)
    a_f32 = consts.tile([P, NG], F32, tag="a_f32")
    nc.vector.tensor_copy(a_f32[:], a_i32[:])
    b_f32 = consts.tile([P, NG], F32, tag="b_f32")
    nc.vector.tensor_copy(b_f32[:], b_i32[:])

    # 7) histogram: hist2d[a,b] += eqA_i^T @ eqB_i per 128-row group.
    hist_ps = psum.tile([HA, HB], F32, tag="hist_ps")
    for i in range(NG):
        eqA = eq_pool.tile([P, HA], F16, tag="eqA", name=f"eqA_{i}")
        nc.vector.tensor_scalar(out=eqA[:], in0=iota128[:, 0:HA],
                                scalar1=a_f32[:, i:i + 1], scalar2=None,
                                op0=Alu.is_equal)
        eqB = eq_pool.tile([P, HB], F16, tag="eqB", name=f"eqB_{i}")
        nc.vector.tensor_scalar(out=eqB[:], in0=iota128[:],
                                scalar1=b_f32[:, i:i + 1], scalar2=None,
                                op0=Alu.is_equal)
        nc.tensor.matmul(hist_ps[:], eqA[:], eqB[:],
                         start=(i == 0), stop=(i == NG - 1))

    # hist2d drains mid-stream: ACT stage + its own small HWDGE DMA.
    hist_sb = consts.tile([HA, HB], F32, tag="hist_sb")
    nc.scalar.copy(hist_sb[:], hist_ps[:])
    nc.sync.dma_start(out=hist_d[:, :], in_=hist_sb[:])

    # 8) DMA-paced colsum matmuls: 4 fp8 chunks per full tile, banks
    # alternate so bank0 = classes 0:500, bank1 = 500:1000.
    cs_ps = [psum.tile([4, CH], F32, tag=f"cs_ps{h}", name=f"cs_ps{h}")
             for h in range(2)]
    for k in range(NBF):
        for q in range(2 * J):
            sl = slice(q * CH, (q + 1) * CH)
            nc.tensor.matmul(cs_ps[q % 2][:], ones4[:], pf8s[k][:, sl],
                             start=(k == 0 and q < 2), stop=False)

    # 9) focal chain, fully hidden mid-stream (pt lands ~mid-stream):
    # pl = [pt | ln pt], focal[p] = sum_m (pt-1)*ln(pt), PE-fold to [1,1].
    pl = consts.tile([P, 2 * NG], F32, tag="pl")
    nc.scalar.copy(pl[:, 0:NG], pt_all[:])
    nc.scalar.activation(pl[:, NG:2 * NG], pt_all[:], Act.Ln)
    junk2 = consts.tile([P, NG], F32, tag="junk2")
    focal = consts.tile([P, 1], F32, tag="focal")
    nc.vector.scalar_tensor_tensor(
        out=junk2[:], in0=pl[:, 0:NG], scalar=1.0, in1=pl[:, NG:2 * NG],
        op0=Alu.subtract, op1=Alu.mult, accum_out=focal[:],
    )

    # 10) tail: two half-tiles -> 2 matmuls each (bank0 closes on the
    # last half's q0); staging copies are [4,500] partition-parallel.
    out_sb = consts.tile([4, OUT_W], F32, tag="out_sb")
    for h in range(2):
        for q in range(2):
            sl = slice(q * CH, (q + 1) * CH)
            nc.tensor.matmul(cs_ps[q][:], ones4[:], pf8s[NBF + h][:, sl],
                             start=False, stop=(h == 1))
        if h == 1:
            nc.vector.tensor_copy(out_sb[:, 0:CH], cs_ps[0][:])
    fc_ps = psum.tile([1, 1], F32, tag="fc_ps")
    nc.tensor.matmul(fc_ps[:], ones_f32[:], focal[:], start=True, stop=True)

    nc.scalar.copy(out_sb[:, CH:2 * CH], cs_ps[1][:])
    nc.scalar.copy(out_sb[0:1, 2 * CH:OUT_W], fc_ps[:])
    nc.sync.dma_start(out=out_d[:, :], in_=out_sb[:])


def _split_multi_waits(nc):
    """The walrus build in this env encodes at most ONE sync wait per
    instruction (newer Tile emits several, e.g. on its tail drain). Hoist
    extra waits onto EventSemaphore carrier instructions inserted just
    before, on the same engine — same-engine program order makes this
    semantically identical."""
    n = 0
    for f in nc.m.functions:
        for blk in f.blocks:
            il = blk.instructions
            i = 0
            while i < len(il):
                inst = il[i]
                si = inst.sync_info
                ws = list(si.on_wait) if si is not None else []
                if len(ws) > 1:
                    for w in ws[:-1]:
                        ev = mybir.InstEventSemaphore(
                            name=f"I-waitsplit-{n}", ins=[], outs=[])
                        n += 1
                        ev.engine = inst.engine
                        ev.sync_info = mybir.SyncInfo(on_wait=[w], on_update=[])
                        il.insert(i, ev)
                        i += 1
                    inst.sync_info = mybir.SyncInfo(
                        on_wait=[ws[-1]], on_update=list(si.on_update))
                i += 1


def _compact_sem_ids(nc, base=3):
    """Tile/bass allocate semaphore ids from ~151 up; remap every semaphore
    this program touches down to [base, base+n) so the program sits inside
    a small --max-sem-num cap. ids 0-2 stay free for the compiler's own
    barriers."""
    def insts():
        for f in nc.m.functions:
            for b in f.blocks:
                yield from b.instructions

    used = set()
    for inst in insts():
        si = inst.sync_info
        if si:
            for w in list(si.on_wait):
                if w.sync_type == "semaphore":
                    used.add(w.id)
            for u in list(si.on_update):
                if u.sync_type == "semaphore":
                    used.add(u.id)
    m = {old: base + i for i, old in enumerate(sorted(used))}
    for inst in insts():
        si = inst.sync_info
        if si:
            ws, us = list(si.on_wait), list(si.on_update)
            changed = False
            for w in ws:
                if w.sync_type == "semaphore" and w.id in m:
                    w.id = m[w.id]
                    changed = True
            for u in us:
                if u.sync_type == "semaphore" and u.id in m:
                    u.id = m[u.id]
                    changed = True
            if changed:
                inst.sync_info = mybir.SyncInfo(on_wait=ws, on_update=us)
        if (type(inst).__name__ == "InstISA"
                and getattr(inst, "op_name", "") == "EVENT_SEMAPHORE_RANGE_CLEAR"):
            d = inst.ant_dict
            ids = [m[x] for x in range(d["range_first"], d["range_last"] + 1)
                   if x in m]
            nf, nl = (min(ids), max(ids)) if ids else (base, base)
            d["range_first"], d["range_last"] = nf, nl
            v = list(inst.instr)
            v[13], v[14] = nf, nl
            inst.instr = v
            inst.ant_dict = d


_cached_nc = {}


def build_nc(split_waits=True):
    global _cached_nc
    if split_waits in _cached_nc:
        return _cached_nc[split_waits]
    from contextlib import ExitStack

    nc = bass.Bass("TRN2", dynamic_dma_scratch_size=131072)
    probs_d = nc.dram_tensor("probs", [BC, C], F32, kind="ExternalInput").ap()
    targ_d = nc.dram_tensor("targets", [BC], I32, kind="ExternalInput").ap()
    out_d = nc.dram_tensor("out_all", [4, OUT_W], F32, kind="ExternalOutput").ap()
    hist_d = nc.dram_tensor("out_hist", [HA, HB], F32, kind="ExternalOutput").ap()

    with tile.TileContext(nc) as tc:
        with ExitStack() as ctx:
            emit_kernel(ctx, tc, probs_d, targ_d, out_d, hist_d)
    if split_waits:
        _split_multi_waits(nc)
    _compact_sem_ids(nc)
    _cached_nc[split_waits] = nc
    return nc


def make_in_maps(probs, targets):
    probs = np.ascontiguousarray(np.asarray(probs), dtype=np.float32)
    targets = np.asarray(targets).astype(np.int32)
    assert probs.shape == (B, C) and targets.shape == (B,)
    return [
        {
            "probs": probs[k * BC:(k + 1) * BC],
            "targets": np.ascontiguousarray(targets[k * BC:(k + 1) * BC]),
        }
        for k in range(NCORES)
    ]


def combine(results):
    cs = np.zeros(C, np.float64)
    hs = np.zeros(C, np.float64)
    fc = 0.0
    for r in results:
        rows = r["out_all"].reshape(4, OUT_W).astype(np.float64)
        cs[0:CH] += rows[:, 0:CH].sum(axis=0)
        cs[CH:C] += rows[:, CH:C].sum(axis=0)
        fc += rows[0, C]
        hs += r["out_hist"].reshape(HA * HB).astype(np.float64)[0:C]
    loss_cls = fc / B
    loss_cal = float(np.mean(np.abs(cs / B - hs / B)))
    return np.asarray(loss_cls + 1.0 * loss_cal, dtype=np.float32)


def run_spmd(probs, targets, **kwargs):
    nc = build_nc()
    in_maps = make_in_maps(probs, targets)
    return run_bass_kernel_spmd(nc, in_maps, list(range(NCORES)), **kwargs)


def kernel(probs, targets):
    res = run_spmd(probs, targets)
    return combine(res.results)


# revision 15
# speedup vs baseline: 1.2200x; 1.0118x over previous
"""FocalLoss + MDCA loss kernel for TRN2, 8-core data-parallel. v6.

reference:
    loss_cls = mean_i[-(1-pt_i) * log(pt_i)],  pt_i = probs[i, targets[i]]
    loss_cal = mean_c |mean_i probs[i,c] - count_c/B|
    out = loss_cls + loss_cal        (GAMMA=1, BETA=1)

Strategy: shard batch (16384) across 8 cores (2048 rows each). The SWDGE
stream of the 8.2 MB probs shard dominates; measurements show it is DMA-
engine/write-side paced (~180 GB/s write-side for cast DMAs), so the colsum
copy converts f32 -> f8e5 (e5m2) in flight: half the write bytes of fp16.
probs ~1e-3 sit in e5m2's normal range (min normal 2^-14); the ~12%/elem
quantization averages out over 2048 rows and only touches loss_cal
(~6% of the total loss) -> ~1e-3 relative error on the loss, 20x inside
the 2e-2 gate. pt for the focal term comes from an EXACT fp32 indirect
gather, and the histogram from exact 0/1 fp16 one-hots, so neither is
touched by fp8.

  - probs: seven [128,2000] f8e5 big-tiles (256 rows: partition p holds
    rows 256k+2p, 256k+2p+1; 8000 B contiguous read descriptors) + TWO
    [128,1000] half-tiles for the last 256 rows so the final DMA gates
    only 2 tail matmuls. gpsimd emits ONLY DMAs (plus one tiny iota):
    k0,k1,k2, rowbase-iota, pt-gather, k3..k7b - the queue never starves
    and the gather's 2048 tiny descriptors execute mid-stream.
  - column sums: ones^T @ tile chunks, 32 fp8 matmuls into 2 PSUM banks.
  - histogram: rank-2 factorization c = 128a+b. eqA[p,:]=(iota8==a(t)),
    eqB[p,:]=(iota128==b(t)); PSUM hist2d[8,128] += eqA^T @ eqB per
    128-row group: 16 tiny fp16 matmuls that depend only on the early
    HWDGE targets load; they warm the PE clock and drain mid-stream via
    their own [8,128] output DMA. Exact counts.
  - pt: ONE indirect gather (offsets = 1000*row + t built from a tiny
    gpsimd iota + DVE add), exact fp32. It lands mid-stream, so the whole
    focal chain (ACT [pt|ln pt], DVE (pt-1)*ln(pt) row-fold) is hidden;
    ones_f32^T @ focal folds partitions into a [1,1] PSUM scalar.
  - targets arrive pre-arranged via ONE strided HWDGE descriptor set:
    t_bt[p, 2k+j] = targets[256k+2p+j] (no PE transpose, no identity).
  - tail after the last half-tile packet: 2 fp8 matmuls -> stage
    [colsum | focal] -> one [1,1001] DMA. Host combines cores: colsum /
    hist2d all-reduce + focal sum, then the loss formula.

The walrus build in this env encodes at most ONE sync wait per instruction;
_split_multi_waits post-processes the scheduled program to hoist extra waits
onto same-engine EventSemaphore carriers. _compact_sem_ids densely remaps
semaphores to ids 3.. and --max-sem-num caps the allocator.
"""

import numpy as np

import concourse.bass as bass
import concourse.bass_utils as _bu
import concourse.mybir as mybir
import concourse.tile as tile
from concourse.bass_utils import run_bass_kernel_spmd

if not getattr(_bu.bir_verify_and_optimise, "_sem_capped", False):
    _orig_bvo = _bu.bir_verify_and_optimise

    def _patch_neff_rtsem(neff_path):
        """Optionally raise def.json's runtime_semaphore_count. The runtime's
        end-of-NEFF sweep clears every semaphore id EXCEPT the first
        runtime_semaphore_count — raising it shrinks the ~250-instruction
        per-id clear loop the runtime appends to the engine streams. Our
        program's own EVENT_SEMAPHORE_RANGE_CLEAR already zeroes the ids it
        used, so a re-execution still starts clean."""
        import io as _io
        import os as _os
        import tarfile as _tarfile
        import tempfile as _tempfile

        import orjson as _orjson

        from concourse.neff import make_deterministic_neff_header

        val = _os.environ.get("KERNEL_RT_SEM_COUNT", "")
        if not val:
            return
        with _tempfile.TemporaryDirectory() as rd:
            with open(neff_path, "rb") as f:
                old_header = f.read(1024)
                with _tarfile.open(fileobj=f, mode="r") as t:
                    t.extractall(rd)
            p = f"{rd}/sg00/def.json"
            d = _orjson.loads(open(p, "rb").read())
            d["runtime_semaphore_count"] = int(val)
            open(p, "wb").write(_orjson.dumps(d))
            buf = _io.BytesIO()

            def _reset(ti):
                ti.mtime = 0
                ti.uid = 0
                ti.gid = 0
                ti.uname = "nobody"
                ti.gname = "nobody"
                return ti

            with _tarfile.open(fileobj=buf, mode="w") as t:
                t.add(rd, arcname=".", filter=_reset)
            data = buf.getvalue()
            header = make_deterministic_neff_header(
                old_neff_header=old_header, new_neff_data=data)
        with open(neff_path, "wb") as f:
            f.write(header + data)

    def _bvo_capped(*args, **kwargs):
        import concourse.bass_utils as bu

        orig_run = bu.run_command

        def run_with_cap(cmd, **kw):
            if any("codegen" in str(c) for c in cmd):
                cmd = list(cmd) + ["--max-sem-num=32"]
                import os as _os
                extra = _os.environ.get("KERNEL_WALRUS_EXTRA", "")
                if extra:
                    cmd = cmd + extra.split()
            return orig_run(cmd, **kw)

        bu.run_command = run_with_cap
        try:
            ret = _orig_bvo(*args, **kwargs)
        finally:
            bu.run_command = orig_run
        if isinstance(ret, str):
            try:
                _patch_neff_rtsem(ret)
            except Exception as e:
                print(f"neff rtsem patch skipped: {e}")
        return ret

    _bvo_capped._sem_capped = True
    _bu.bir_verify_and_optimise = _bvo_capped

B, C = 16384, 1000
NCORES = 8
BC = B // NCORES  # 2048 rows per core
P = 128
NBF = 7           # full big-tiles per core: [128, 2000], 256 rows each
J = 2             # rows per partition per full big-tile
W = J * C         # 2000 columns per big-tile
NG = 16           # 128-row groups per core (pt / hist granularity)
CH = 500          # matmul chunk free-dim (PSUM bank = 512 fp32)
NWU = 4           # PE warm-up matmuls
OUT_W = 1001      # [colsum 0:1000 | focal_sum]
HA, HB = 8, 128   # hist2d factorization: class c = 128*a + b

F32 = mybir.dt.float32
F16 = mybir.dt.float16
F8 = mybir.dt.float8e5
I32 = mybir.dt.int32


def emit_kernel(ctx, tc, probs_d, targ_d, out_d, hist_d):
    nc = tc.nc
    Alu = mybir.AluOpType
    Act = mybir.ActivationFunctionType

    consts = ctx.enter_context(tc.tile_pool(name="consts", bufs=1))
    probs_pool = ctx.enter_context(tc.tile_pool(name="probs_pool", bufs=NBF + 2))
    eq_pool = ctx.enter_context(tc.tile_pool(name="eq_pool", bufs=NG))
    psum = ctx.enter_context(tc.tile_pool(name="psum", bufs=1, space="PSUM"))

    # 1) targets first: ONE strided HWDGE load lands t_bt[p, 2k+j] =
    # targets[256k+2p+j] (descriptor: 8 chunks x 8 B, stride 1 KiB).
    t_bt_i32 = consts.tile([P, NG], I32, tag="t_bt_i32")
    nc.sync.dma_start(
        out=t_bt_i32[:],
        in_=targ_d.rearrange("(k p j) -> p k j", k=NBF + 1, p=P, j=J),
    )

    # 2) probs stream, f32 -> f8e5 in flight (write-side is the DMA pacing
    # constraint; e5m2 halves it vs fp16).
    def load_full(k):
        pf8 = probs_pool.tile([P, W], F8, tag="pf8", name=f"pf8_{k}")
        nc.gpsimd.dma_start(
            out=pf8[:],
            in_=probs_d[k * J * P:(k + 1) * J * P, :].rearrange(
                "(p j) c -> p (j c)", p=P, j=J),
        )
        return pf8

    def load_half(h):
        pf8 = probs_pool.tile([P, C], F8, tag="pf8", name=f"pf8_7{'ab'[h]}")
        nc.gpsimd.dma_start(
            out=pf8[:],
            in_=probs_d[NBF * J * P:(NBF + 1) * J * P, :].rearrange(
                "(p j) c -> p (j c)", p=P, j=J)[:, h * C:(h + 1) * C],
        )
        return pf8

    pf8s = [load_full(k) for k in range(3)]

    # 3) rowbase iota + pt gather emissions sit between k2 and k3 so the
    # 2048 tiny descriptors execute mid-stream, and offs (needs the HWDGE
    # targets, landing ~2 us in) is ready just in time.
    rowidx = consts.tile([P, NG], I32, tag="rowidx")
    nc.gpsimd.iota(rowidx[:], pattern=[[J * P, NBF + 1], [1, J]], base=0,
                   channel_multiplier=J)
    offs = consts.tile([P, NG], I32, tag="offs")
    nc.vector.tensor_scalar(out=offs[:], in0=rowidx[:], scalar1=float(C),
                            scalar2=None, op0=Alu.mult)
    nc.vector.tensor_tensor(out=offs[:], in0=offs[:], in1=t_bt_i32[:],
                            op=Alu.add)
    pt_all = consts.tile([P, NG], F32, tag="pt_all")
    nc.gpsimd.indirect_dma_start(
        out=pt_all[:], out_offset=None,
        in_=probs_d.rearrange("a b -> (a b)")[:, None],
        in_offset=bass.IndirectOffsetOnAxis(ap=offs[:], axis=0),
    )

    pf8s += [load_full(k) for k in range(3, NBF)]
    pf8s += [load_half(0), load_half(1)]

    # 4) constants on DVE: iota128 by prefix-scan, ones in three dtypes.
    ones128 = consts.tile([P, HB], F16, tag="ones128")
    nc.vector.memset(ones128[:], 1.0)
    ones_f8 = consts.tile([P, 1], F8, tag="ones_f8")
    nc.vector.memset(ones_f8[:], 1.0)
    ones_f32 = consts.tile([P, 1], F32, tag="ones_f32")
    nc.vector.memset(ones_f32[:], 1.0)
    wu_f8 = consts.tile([P, CH], F8, tag="wu_f8")
    nc.vector.memset(wu_f8[:], 0.5)
    iota128 = consts.tile([P, HB], F16, tag="iota128")
    nc.vector.tensor_tensor_scan(
        out=iota128[:], data0=ones128[:], data1=ones128[:],
        initial=-1.0, op0=Alu.add, op1=Alu.bypass,
    )
    # block-ones lhsT: ones4[p, c] = (c == p>>5) so colsum lands as [4,500]
    # per bank (partition-parallel staging copies, host sums the 4 rows).
    chi5 = consts.tile([P, 1], I32, tag="chi5")
    nc.vector.tensor_scalar(out=chi5[:], in0=rowidx[:, 0:1], scalar1=6,
                            scalar2=None, op0=Alu.arith_shift_right)
    chi5f = consts.tile([P, 1], F32, tag="chi5f")
    nc.vector.tensor_copy(chi5f[:], chi5[:])
    ones4 = consts.tile([P, 4], F8, tag="ones4")
    nc.vector.tensor_scalar(out=ones4[:], in0=iota128[:, 0:4],
                            scalar1=chi5f[:], scalar2=None, op0=Alu.is_equal)

    # 5) PE warm-up on the fp8 path while the first tiles stream in.
    wu_ps = psum.tile([1, CH], F32, tag="wu_ps")
    for w in range(NWU):
        nc.tensor.matmul(wu_ps[:], ones_f8[:], wu_f8[:],
                         start=(w == 0), stop=(w == NWU - 1))

    # 6) a = t>>7, b = t&127 (f32 for the eq compares).
    a_i32 = consts.tile([P, NG], I32, tag="a_i32")
    nc.vector.tensor_scalar(out=a_i32[:], in0=t_bt_i32[:], scalar1=7,
                            scalar2=None, op0=Alu.arith_shift_right)
    b_i32 = consts.tile([P, NG], I32, tag="b_i32")
    nc.vector.tensor_scalar(out=b_i32[:], in0=t_bt_i32[:], scalar1=127,
                            scalar2=None, op0=Alu.bitwise_and)
    a_f32 = consts.tile([P, NG], F32, tag="a_f32")
    nc.vector.tensor_copy(a_f32[:], a_i32[:])
    b_f32 = consts.tile([P, NG], F32, tag="b_f32")
    nc.vector.tensor_copy(b_f32[:], b_i32[:])

    # 7) histogram: hist2d[a,b] += eqA_i^T @ eqB_i per 128-row group.
    hist_ps = psum.tile([HA, HB], F32, tag="hist_ps")
    for i in range(NG):
        eqA = eq_pool.tile([P, HA], F16, tag="eqA", name=f"eqA_{i}")
        nc.vector.tensor_scalar(out=eqA[:], in0=iota128[:, 0:HA],
                                scalar1=a_f32[:, i:i + 1], scalar2=None,
                                op0=Alu.is_equal)
        eqB = eq_pool.tile([P, HB], F16, tag="eqB", name=f"eqB_{i}")
        nc.vector.tensor_scalar(out=eqB[:], in0=iota128[:],
                                scalar1=b_f32[:, i:i + 1], scalar2=None,
                                op0=Alu.is_equal)
        nc.tensor.matmul(hist_ps[:], eqA[:], eqB[:],
                         start=(i == 0), stop=(i == NG - 1))

    # hist2d drains mid-stream: ACT stage + its own small HWDGE DMA.
    hist_sb = consts.tile([HA, HB], F32, tag="hist_sb")
    nc.scalar.copy(hist_sb[:], hist_ps[:])
    nc.sync.dma_start(out=hist_d[:, :], in_=hist_sb[:])

    # 8) DMA-paced colsum matmuls: 4 fp8 chunks per full tile, banks
    # alternate so bank0 = classes 0:500, bank1 = 500:1000.
    cs_ps = [psum.tile([4, CH], F32, tag=f"cs_ps{h}", name=f"cs_ps{h}")
             for h in range(2)]
    for k in range(NBF):
        for q in range(2 * J):
            sl = slice(q * CH, (q + 1) * CH)
            nc.tensor.matmul(cs_ps[q % 2][:], ones4[:], pf8s[k][:, sl],
                             start=(k == 0 and q < 2), stop=False)

    # 9) focal chain, fully hidden mid-stream (pt lands ~mid-stream):
    # pl = [pt | ln pt], focal[p] = sum_m (pt-1)*ln(pt), PE-fold to [1,1].
    pl = consts.tile([P, 2 * NG], F32, tag="pl")
    nc.scalar.copy(pl[:, 0:NG], pt_all[:])
    nc.scalar.activation(pl[:, NG:2 * NG], pt_all[:], Act.Ln)
    junk2 = consts.tile([P, NG], F32, tag="junk2")
    focal = consts.tile([P, 1], F32, tag="focal")
    nc.vector.scalar_tensor_tensor(
        out=junk2[:], in0=pl[:, 0:NG], scalar=1.0, in1=pl[:, NG:2 * NG],
        op0=Alu.subtract, op1=Alu.mult, accum_out=focal[:],
    )

    # 10) tail: two half-tiles -> 2 matmuls each (bank0 closes on the
    # last half's q0); staging copies are [4,500] partition-parallel.
    out_sb = consts.tile([4, OUT_W], F32, tag="out_sb")
    for h in range(2):
        for q in range(2):
            sl = slice(q * CH, (q + 1) * CH)
            nc.tensor.matmul(cs_ps[q][:], ones4[:], pf8s[NBF + h][:, sl],
                             start=False, stop=(h == 1))
        if h == 1:
            nc.vector.tensor_copy(out_sb[:, 0:CH], cs_ps[0][:])
    fc_ps = psum.tile([1, 1], F32, tag="fc_ps")
    nc.tensor.matmul(fc_ps[:], ones_f32[:], focal[:], start=True, stop=True)

    nc.scalar.copy(out_sb[:, CH:2 * CH], cs_ps[1][:])
    nc.scalar.copy(out_sb[0:1, 2 * CH:OUT_W], fc_ps[:])
    nc.sync.dma_start(out=out_d[:, :], in_=out_sb[:])


def _split_multi_waits(nc):
    """The walrus build in this env encodes at most ONE sync wait per
    instruction (newer Tile emits several, e.g. on its tail drain). Hoist
    extra waits onto EventSemaphore carrier instructions inserted just
    before, on the same engine — same-engine program order makes this
    semantically identical."""
    n = 0
    for f in nc.m.functions:
        for blk in f.blocks:
            il = blk.instructions
            i = 0
            while i < len(il):
                inst = il[i]
                si = inst.sync_info
                ws = list(si.on_wait) if si is not None else []
                if len(ws) > 1:
                    for w in ws[:-1]:
                        ev = mybir.InstEventSemaphore(
                            name=f"I-waitsplit-{n}", ins=[], outs=[])
                        n += 1
                        ev.engine = inst.engine
                        ev.sync_info = mybir.SyncInfo(on_wait=[w], on_update=[])
                        il.insert(i, ev)
                        i += 1
                    inst.sync_info = mybir.SyncInfo(
                        on_wait=[ws[-1]], on_update=list(si.on_update))
                i += 1


def _compact_sem_ids(nc, base=3):
    """Tile/bass allocate semaphore ids from ~151 up; remap every semaphore
    this program touches down to [base, base+n) so the program sits inside
    a small --max-sem-num cap. ids 0-2 stay free for the compiler's own
    barriers."""
    def insts():
        for f in nc.m.functions:
            for b in f.blocks:
                yield from b.instructions

    used = set()
    for inst in insts():
        si = inst.sync_info
        if si:
            for w in list(si.on_wait):
                if w.sync_type == "semaphore":
                    used.add(w.id)
            for u in list(si.on_update):
                if u.sync_type == "semaphore":
                    used.add(u.id)
    m = {old: base + i for i, old in enumerate(sorted(used))}
    for inst in insts():
        si = inst.sync_info
        if si:
            ws, us = list(si.on_wait), list(si.on_update)
            changed = False
            for w in ws:
                if w.sync_type == "semaphore" and w.id in m:
                    w.id = m[w.id]
                    changed = True
            for u in us:
                if u.sync_type == "semaphore" and u.id in m:
                    u.id = m[u.id]
                    changed = True
            if changed:
                inst.sync_info = mybir.SyncInfo(on_wait=ws, on_update=us)
        if (type(inst).__name__ == "InstISA"
                and getattr(inst, "op_name", "") == "EVENT_SEMAPHORE_RANGE_CLEAR"):
            d = inst.ant_dict
            ids = [m[x] for x in range(d["range_first"], d["range_last"] + 1)
                   if x in m]
            nf, nl = (min(ids), max(ids)) if ids else (base, base)
            d["range_first"], d["range_last"] = nf, nl
            v = list(inst.instr)
            v[13], v[14] = nf, nl
            inst.instr = v
            inst.ant_dict = d


_cached_nc = {}


def build_nc(split_waits=True):
    global _cached_nc
    if split_waits in _cached_nc:
        return _cached_nc[split_waits]
    from contextlib import ExitStack

    nc = bass.Bass("TRN2", dynamic_dma_scratch_size=131072)
    probs_d = nc.dram_tensor("probs", [BC, C], F32, kind="ExternalInput").ap()
    targ_d = nc.dram_tensor("targets", [BC], I32, kind="ExternalInput").ap()
    out_d = nc.dram_tensor("out_all", [4, OUT_W], F32, kind="ExternalOutput").ap()
    hist_d = nc.dram_tensor("out_hist", [HA, HB], F32, kind="ExternalOutput").ap()

    with tile.TileContext(nc) as tc:
        with ExitStack() as ctx:
            emit_kernel(ctx, tc, probs_d, targ_d, out_d, hist_d)
    if split_waits:
        _split_multi_waits(nc)
    _compact_sem_ids(nc)
    _cached_nc[split_waits] = nc
    return nc


def make_in_maps(probs, targets):
    probs = np.ascontiguousarray(np.asarray(probs), dtype=np.float32)
    targets = np.asarray(targets).astype(np.int32)
    assert probs.shape == (B, C) and targets.shape == (B,)
    return [
        {
            "probs": probs[k * BC:(k + 1) * BC],
            "targets": np.ascontiguousarray(targets[k * BC:(k + 1) * BC]),
        }
        for k in range(NCORES)
    ]


def combine(results):
    cs = np.zeros(C, np.float64)
    hs = np.zeros(C, np.float64)
    fc = 0.0
    for r in results:
        rows = r["out_all"].reshape(4, OUT_W).astype(np.float64)
        cs[0:CH] += rows[:, 0:CH].sum(axis=0)
        cs[CH:C] += rows[:, CH:C].sum(axis=0)
        fc += rows[0, C]
        hs += r["out_hist"].reshape(HA * HB).astype(np.float64)[0:C]
    loss_cls = fc / B
    loss_cal = float(np.mean(np.abs(cs / B - hs / B)))
    return np.asarray(loss_cls + 1.0 * loss_cal, dtype=np.float32)


def run_spmd(probs, targets, **kwargs):
    nc = build_nc()
    in_maps = make_in_maps(probs, targets)
    return run_bass_kernel_spmd(nc, in_maps, list(range(NCORES)), **kwargs)


def kernel(probs, targets):
    res = run_spmd(probs, targets)
    return combine(res.results)
